# revision 1
# baseline (speedup 1.0000x reference)
"""Multi-head attention (B=4, S=2048, D=1024, H=16) on 8 TRN2 NeuronCores, v2.

Sharding: core c <- (batch b = c // 2, head-group g = c % 2); head-group =
8 heads = 512 projection dims. Per core:

    QT = (q[b] @ Wq_g)^T   [512, S]  (bf16, head-major partitions)
    KT = (k[b] @ Wk_g)^T   [512, S]
    V  =  v[b] @ Wv_g      [S, 8*65] (64 data cols + ones col per head)
    attention per head pair, q-tiles of 512:
        scoresT = K_h Q_h^T / 8 -> exp (ScalarE, scale folded) -> P^T bf16
        AV with P^T chunks [128k,128q] STATIONARY and V [128k,65] MOVING:
        psum av[q, 65] accumulates over k-chunks; col 64 = denominator.
        normalize: DVE reciprocal + per-partition-scalar mul -> av_n bf16
        PE transpose (identity matmul) -> AOT^T [hd, q] psum -> DVE -> SBUF
    outT_partial = Wo_g^T @ attn_outT  [1024, S] (f32)

Host: out[b] = (outT_{b,0} + outT_{b,1})^T + bo.

The moving-operand free size is what the PE pays for, so AV costs 65/512 of
the v1 formulation. Projections and output projection are emitted as small
"filler" quanta interleaved into the attention loop so the PE never idles
while ScalarE grinds exp.
"""

import numpy as np
import ml_dtypes

B, S, D, H = 4, 2048, 1024, 16
HD = 64
G = D // 2          # per-core head-group width = 512
NH = G // HD        # heads per core = 8
PAIRS = NH // 2
SCALE = 1.0 / np.sqrt(HD)

_CACHE = {}


def _split_multiwaits(nc, cap=1):
    """The walrus build in this container rejects instructions carrying more
    than `cap` sem waits (Tile's tail drain has 3). Move extra waits onto
    no-op instructions inserted just before, on the same engine."""
    import concourse.mybir as mybir

    n = 0
    for func in nc.m.functions:
        for blk in func.blocks:
            insts = list(blk.instructions)
            new_insts = []
            changed = False
            for inst in insts:
                si = inst.sync_info
                if si is not None and si.on_wait and len(si.on_wait) > cap:
                    waits = list(si.on_wait)
                    extra, keep = waits[:-cap], waits[-cap:]
                    for j, w in enumerate(extra):
                        nop = mybir.InstNoOp(
                            name=f"{inst.name}-wsplit{j}",
                            sync_info=mybir.SyncInfo(on_wait=[w], on_update=[]),
                            engine=inst.engine,
                            bass_nofuse=True,
                        )
                        new_insts.append(nop)
                        n += 1
                    inst.sync_info = mybir.SyncInfo(
                        on_wait=keep, on_update=list(si.on_update)
                    )
                    changed = True
                new_insts.append(inst)
            if changed:
                blk.instructions = new_insts
    return n


def _strip_ldweights(nc):
    """Drop Tile's separate Ldweights instructions; walrus handles
    self-loading matmuls. LDWs that carry sems become NoOps."""
    import concourse.mybir as mybir

    n = 0
    for func in nc.m.functions:
        for blk in func.blocks:
            insts = list(blk.instructions)
            new_insts = []
            changed = False
            for inst in insts:
                if type(inst).__name__ == "InstLdweights":
                    si = inst.sync_info
                    if si is not None and (si.on_wait or si.on_update):
                        new_insts.append(mybir.InstNoOp(
                            name=inst.name,
                            sync_info=mybir.SyncInfo(
                                on_wait=list(si.on_wait),
                                on_update=list(si.on_update)),
                            engine=inst.engine,
                            bass_nofuse=True,
                        ))
                    n += 1
                    changed = True
                    continue
                if type(inst).__name__ == "InstMatmult":
                    inst.ldweights = True
                new_insts.append(inst)
            if changed:
                blk.instructions = new_insts
    return n


def build_mha_nc(dbg=None):
    import concourse.bass as bass
    import concourse.mybir as mybir
    import concourse.tile as tile

    dt = mybir.dt
    f32 = dt.float32
    bf16 = dt.bfloat16
    Exp = mybir.ActivationFunctionType.Exp

    s, d, g = S, D, G
    mch = g // 128        # head-pair chunks = 4
    kch = d // 128        # contraction chunks over D = 8
    sch = s // 128        # S chunks (k-chunks in attention) = 16
    qw = 512
    nqw = s // qw         # attention q-tiles per pair = 4
    sn = s // 512         # 512-wide blocks over S = 4

    nc = bass.Bass("TRN2", target_bir_lowering=False, debug=False)

    qT = nc.declare_dram_parameter("qT", [d, s], bf16, isOutput=False)
    kT = nc.declare_dram_parameter("kT", [d, s], bf16, isOutput=False)
    vT = nc.declare_dram_parameter("vT", [d, s], bf16, isOutput=False)
    Wq = nc.declare_dram_parameter("Wq", [d, g], bf16, isOutput=False)
    Wk = nc.declare_dram_parameter("Wk", [d, g], bf16, isOutput=False)
    Wv = nc.declare_dram_parameter("Wv", [d, g], bf16, isOutput=False)
    Wo = nc.declare_dram_parameter("Wo", [g, d], bf16, isOutput=False)
    bq = nc.declare_dram_parameter("bq", [128, mch], f32, isOutput=False)
    bk = nc.declare_dram_parameter("bk", [128, mch], f32, isOutput=False)
    bv = nc.declare_dram_parameter("bv", [1, g], bf16, isOutput=False)
    ident = nc.declare_dram_parameter("ident", [128, 128], bf16,
                                      isOutput=False)
    outT = nc.declare_dram_parameter("outT", [d, s], bf16, isOutput=True)

    with tile.TileContext(nc) as tc:
        with (
            tc.tile_pool(name="const", bufs=1) as const,
            tc.tile_pool(name="acts", bufs=1) as acts,
            tc.tile_pool(name="inT", bufs=1) as inT_pool,
            tc.tile_pool(name="pTp", bufs=2) as pTp,
            tc.tile_pool(name="nrm", bufs=2) as nrm,
            tc.tile_pool(name="outsb", bufs=5) as outp,
            tc.tile_pool(name="ps", bufs=2,
                         space=bass.MemorySpace.PSUM) as ps,
        ):
            # ---- constants ----
            # All DMAs ride the sync (HWDGE) queue: fixed ~625ns/instruction
            # with no descriptor-count penalty, and SP issues in emission
            # order, so DMA_ENGINES (which serializes every transfer at
            # ~360B/ns) processes exactly the order written below. Weights
            # are interleaved into the input stream at their point of need.
            Wv_sb = const.tile([128, kch, g], bf16)
            Wk_sb = const.tile([128, kch, g], bf16)
            Wq_sb = const.tile([128, kch, g], bf16)
            Wo_sb = const.tile([128, mch, d], bf16)
            bq_sb = const.tile([128, mch], f32)
            bk_sb = const.tile([128, mch], f32)
            ident_sb = const.tile([128, 128], bf16)
            nc.sync.dma_start(bq_sb[:], bq[:])
            nc.sync.dma_start(bk_sb[:], bk[:])
            ones_sb = const.tile([128, 512], bf16)
            nc.vector.memset(ones_sb[:], 1.0)

            # ---- resident activations ----
            QT_sb = acts.tile([128, mch, s], bf16)   # Q^T head-major
            KT_sb = acts.tile([128, mch, s], bf16)
            # V natural [S, nh*65]: per head 64 data cols + a ones column
            # (65th moving col of the AV matmul = softmax denominator).
            V_sb = acts.tile([128, sch, NH * 65], bf16)
            AOT_sb = acts.tile([128, mch, s], bf16)  # attn_out^T
            nc.vector.memset(
                V_sb.rearrange("p s (h c) -> p s h c", c=65)[:, :, :, 64:65],
                1.0)

            # Input tiles, DMA'd in 1024-col half-blocks on the sync queue
            # (HWDGE: fixed ~625ns/instr, no descriptor-count penalty, and
            # the SP queue preserves emission order on DMA_ENGINES, which
            # serializes all transfers at ~360B/ns). Order = consumption
            # order of the prologue + first attention unit.
            vT_t = inT_pool.tile([128, kch, s], bf16, name="vT_t")
            kT_t = inT_pool.tile([128, kch, s], bf16, name="kT_t")
            qT_t = inT_pool.tile([128, kch, s], bf16, name="qT_t")

            def dma_block(src, t, c0, c1):
                # one instruction moves the column block of ALL 8 d-chunks
                # (HWDGE charges ~625ns per instruction, so per-chunk
                # transfers made the early stream DGE-bound)
                nc.sync.dma_start(
                    t[:, :, c0:c1],
                    src.rearrange("(c p) n -> p c n", p=128)[:, :, c0:c1])

            # K and the first Q block lead (they gate the first scores+exp
            # at ~18us); Wv/vT stream in behind them, consumed chunk-by-chunk
            # by V-proj fillers inside unit 0's kc loop. Everything unit 0
            # touches is on-device by ~34us, which pins its last exp.
            nc.sync.dma_start(
                Wk_sb[:], Wk.rearrange("(c p) n -> p c n", p=128))
            dma_block(kT, kT_t, 0, 1024)
            nc.sync.dma_start(
                Wq_sb[:], Wq.rearrange("(c p) n -> p c n", p=128))
            dma_block(qT, qT_t, 0, 512)
            nc.sync.dma_start(
                Wv_sb[:], Wv.rearrange("(c p) n -> p c n", p=128))
            dma_block(vT, vT_t, 0, 512)
            dma_block(vT, vT_t, 512, 1024)
            dma_block(kT, kT_t, 1024, 2048)
            dma_block(vT, vT_t, 1024, 2048)
            dma_block(qT, qT_t, 512, 1024)
            nc.sync.dma_start(ident_sb[:], ident[:])
            dma_block(qT, qT_t, 1024, 2048)
            nc.sync.dma_start(
                Wo_sb[:], Wo.rearrange("(c p) n -> p c n", p=128))

            # ---------------- emission helpers ----------------
            def v_proj_chunk(sc, glo, ghi):
                """V projection for s-chunk sc, g columns [glo, ghi)."""
                w = ghi - glo
                vp = ps.tile([128, 512], f32, tag="px", name="vp")
                for kk in range(kch):
                    nc.tensor.matmul(
                        vp[:, 0:w],
                        vT_t[:, kk, sc * 128:(sc + 1) * 128],
                        Wv_sb[:, kk, glo:ghi],
                        start=(kk == 0), stop=(kk == kch - 1),
                    )
                    yield 0.42 * w
                # bv is folded into bo on the host (it rides through the
                # softmax weighting exactly: sum_k p_k (V+bv) / denom =
                # sum_k p_k V / denom + bv), so no bias matmul here.
                h0 = glo // HD
                nc.vector.tensor_copy(
                    V_sb[:, sc].rearrange(
                        "p (h c) -> p h c", c=65)[:, h0:h0 + w // HD, 0:64],
                    vp[:, 0:w].rearrange("p (h c) -> p h c", c=64))
                yield 0.42 * w

            def qk_proj_chunk(W_sb, x_t, b_sb, dst, m, n0):
                """Q/K projection chunk: m-chunk m (pair), 512-block n0."""
                pp = ps.tile([128, 512], f32, tag="px", name="pp")
                for kk in range(kch):
                    nc.tensor.matmul(
                        pp[:],
                        W_sb[:, kk, m * 128:(m + 1) * 128],
                        x_t[:, kk, n0 * 512:(n0 + 1) * 512],
                        start=(kk == 0), stop=(kk == kch - 1),
                    )
                    yield 213.0
                nc.vector.tensor_scalar_add(
                    dst[:, m, n0 * 512:(n0 + 1) * 512],
                    pp[:], b_sb[:, m:m + 1])
                yield 20.0

            def outproj_chunk(mo, n0, tag, on_act=False):
                """Output projection chunk outT[mo*128:, n0*512:]. Epilogue
                chunks stage through the (by then idle) ScalarE instead of
                the DVE so the copy overlaps the final normalize."""
                op = ps.tile([128, 1024] if tag == "sc" else [128, 512],
                             f32, tag=tag, name="op")
                for kk in range(mch):
                    nc.tensor.matmul(
                        op[:, 0:512],
                        Wo_sb[:, kk, mo * 128:(mo + 1) * 128],
                        AOT_sb[:, kk, n0 * 512:(n0 + 1) * 512],
                        start=(kk == 0), stop=(kk == mch - 1))
                    yield 213.0
                ot = outp.tile([128, 512], bf16, tag="ot", name="ot")
                if on_act:
                    nc.scalar.activation(
                        ot[:], op[:, 0:512],
                        mybir.ActivationFunctionType.Copy)
                else:
                    nc.vector.tensor_copy(ot[:], op[:, 0:512])
                nc.sync.dma_start(
                    outT[mo * 128:(mo + 1) * 128,
                         n0 * 512:(n0 + 1) * 512], ot[:])
                yield 20.0

            # ---------------- prologue ----------------
            # PE p-state warmup: the cost model runs matmuls at half speed
            # until the PE has been continuously busy for 3us. Burn that
            # ramp on dependency-free dummy matmuls while the input DMAs
            # stream, so the real prologue runs at full clock.
            for _ in range(30):
                wt = ps.tile([128, 512], f32, tag="px", name="wt")
                nc.tensor.matmul(wt[:], ones_sb[:, 0:128], ones_sb[:],
                                 start=True, stop=True)
            # Just enough to start attention: K(pair 0) blocks 0-1,
            # Q(0, qtile 0), V pairs 0-1 s-chunk 0. Everything else streams
            # in as fillers inside the attention loop, paced by DMA arrival.
            for n0 in range(2):
                for _ in qk_proj_chunk(Wk_sb, kT_t, bk_sb, KT_sb, 0, n0):
                    pass
            for _ in qk_proj_chunk(Wq_sb, qT_t, bq_sb, QT_sb, 0, 0):
                pass

            # ---------------- filler schedule ----------------
            # (deadline, generator), deadline = u*16 + kc: the filler must be
            # fully EMITTED before that (unit, kc) slot's instructions, since
            # per-engine queues execute in program order (a consumer emitted
            # before its producer would deadlock).
            fillers = []
            for sc in range(sch):
                # V pairs 0-1 s-chunk sc feeds the (lag-1) AV batch of
                # kc == sc, emitted at slot sc+1. Deadline >= 1 so the first
                # chunk's vT DMA wait can't block the unit-0 attention start.
                fillers.append((max(0 * 16 + sc - 1, 1),
                                v_proj_chunk(sc, 0, 256)))
            for n0 in range(2, sn):
                # KT(pair 0) block n0 feeds scores at kc == 4*n0
                fillers.append((0 * 16 + 4 * n0 - 2,
                                qk_proj_chunk(Wk_sb, kT_t, bk_sb,
                                              KT_sb, 0, n0)))
            for sc in range(sch):
                # V pairs 2-3: hard deadline is (u8, kc=sc), but spread the
                # scheduling deadlines across units 2-8 so the windowed pull
                # has steady supply instead of a cluster at u7-8.
                fillers.append((min(32 + 6 * sc, 8 * 16 + sc - 2),
                                v_proj_chunk(sc, 256, 512)))
            # Unit order: pairs 0,1 straight, then pairs 2/3 interleaved so
            # each qtile's last pair (and thus its outproj) completes early.
            units = [(0, 0), (0, 1), (0, 2), (0, 3),
                     (1, 0), (1, 1), (1, 2), (1, 3),
                     (2, 0), (3, 0), (2, 1), (3, 1),
                     (2, 2), (3, 2), (2, 3), (3, 3)]
            uidx = {pt: i for i, pt in enumerate(units)}
            first_u = {0: 0, 1: 4, 2: 8, 3: 9}
            for p in (1, 2, 3):
                for n0 in range(sn):
                    # K(pair p) block n0 needed at (first unit of p, kc=4*n0)
                    fillers.append((first_u[p] * 16 + 4 * n0 - 2,
                                    qk_proj_chunk(Wk_sb, kT_t, bk_sb,
                                                  KT_sb, p, n0)))
            for pr in range(PAIRS):
                for t in range(nqw):
                    if (pr, t) == (0, 0):
                        continue
                    # Q(pr, t) needed at its unit start
                    fillers.append((uidx[(pr, t)] * 16 - 4,
                                    qk_proj_chunk(Wq_sb, qT_t, bq_sb,
                                                  QT_sb, pr, t)))
            fillers.sort(key=lambda x: x[0])
            from collections import deque
            fq = deque(fillers)
            # outproj chunk specs, appended as AOT sn-blocks complete.
            oq = deque()

            # SINGLE-FLIGHT px discipline: the "px" psum ring has 2 slots,
            # so at most ONE chunk generator may be suspended at a time —
            # interleaving two would let a later tile() steal the slot of a
            # chunk whose remaining writes are not yet emitted (silent
            # corruption). `cur` is the one open generator; every other px
            # consumer (transposes, epilogue) must finish it first.
            cur = {"gen": None, "dl": None}

            def finish_cur():
                if cur["gen"] is not None:
                    for c in cur["gen"]:
                        clk["vpe"] += c or 213.0
                    cur["gen"] = None

            def pull(slot, budget):
                """Emit filler quanta. When the virtual clocks are armed,
                pull while the PE frontier stays clear of delaying the next
                scores; otherwise fall back to a fixed quanta budget. The
                slot+32 window keeps supply for later units either way."""
                spent = 0
                while True:
                    if clk["on"]:
                        if clk["vpe"] + 220.0 > clk["vact"] - 1100.0:
                            return
                    elif spent >= budget:
                        return
                    if cur["gen"] is None:
                        if fq and fq[0][0] <= slot + 32:
                            cur["dl"], cur["gen"] = fq.popleft()
                        elif oq:
                            mo, n0 = oq.popleft()
                            cur["gen"] = outproj_chunk(mo, n0, "px")
                            cur["dl"] = None
                        else:
                            return
                    try:
                        clk["vpe"] += next(cur["gen"]) or 213.0
                        spent += 1
                    except StopIteration:
                        cur["gen"] = None

            def drain(slot):
                """Force-finish all fillers with deadline <= slot."""
                while True:
                    if (cur["gen"] is not None and cur["dl"] is not None
                            and cur["dl"] <= slot):
                        finish_cur()
                        continue
                    if fq and fq[0][0] <= slot:
                        finish_cur()  # an open oq-chunk blocks the ring
                        cur["dl"], cur["gen"] = fq.popleft()
                        finish_cur()
                        continue
                    break

            # ---------------- attention ----------------
            # The AV matmul batch for k-chunk kc is deferred by one kc slot
            # (lag-1 software pipeline): by the time the PE reaches it, that
            # chunk's exp has long finished, so the PE never stalls on the
            # Act engine in steady state. The per-qtile normalize/transpose
            # tail rides the same deferral queue with its own lag so the PE
            # transposes land after the DVE normalize has had time to run.
            deferred = []   # (earliest_slot, pe_cost_ns, closure)
            # Virtual engine clocks for self-paced filler emission: vpe is
            # the PE busy frontier, vact the Act frontier. Fillers are
            # pulled only while the PE is behind the point where it would
            # delay the next exp's scores.
            clk = {"vpe": 0.0, "vact": 0.0, "on": False}

            def emit_deferred(slot):
                rest = []
                for es, cost, fn in deferred:
                    if es <= slot:
                        fn()
                        clk["vpe"] += cost
                    else:
                        rest.append((es, cost, fn))
                deferred[:] = rest

            done_t = [0] * nqw
            for u, (pr, t) in enumerate(units):
                if True:
                    hA, hB = 2 * pr, 2 * pr + 1
                    gl = slice(t * qw, (t + 1) * qw)
                    avA = ps.tile([128, 4, 65], f32, tag="av", name="avA")
                    avB = ps.tile([128, 4, 65], f32, tag="av", name="avB")
                    for kc in range(sch):
                        drain(u * 16 + kc)
                        kcs = slice(kc * 128, kc * 128 + 128)
                        scAB = ps.tile([128, 1024], f32, tag="sc",
                                       name="scAB")
                        nc.tensor.matmul(
                            scAB[:, 0:qw], KT_sb[0:64, pr, kcs],
                            QT_sb[0:64, pr, gl], start=True, stop=True)
                        nc.tensor.matmul(
                            scAB[:, qw:2 * qw], KT_sb[64:128, pr, kcs],
                            QT_sb[64:128, pr, gl], start=True, stop=True)
                        pT = pTp.tile([128, 1024], bf16, tag="pT", name="pT")
                        nc.scalar.activation(pT[:], scAB[:], Exp,
                                             scale=float(SCALE))

                        def av_batch(kc=kc, pT=pT, avA=avA, avB=avB,
                                     hA=hA, hB=hB):
                            # One accumulation group per av BANK: start only
                            # on the first matmul touching the bank (it marks
                            # the whole 2KB zero-region pending, so the other
                            # qc slices overwrite-on-first-touch), stop on
                            # the very last.
                            for hoff, h, av in ((0, hA, avA), (qw, hB, avB)):
                                for qc in range(4):
                                    nc.tensor.matmul(
                                        av[:, qc, :],
                                        pT[:, hoff + qc * 128:
                                           hoff + (qc + 1) * 128],
                                        V_sb[:, kc, h * 65:h * 65 + 65],
                                        start=(kc == 0 and qc == 0),
                                        stop=(kc == sch - 1 and qc == 3))
                        slot = u * 16 + kc
                        # clock updates: 2 scores matmuls then this slot's exp
                        clk["vpe"] += 427.0
                        clk["vact"] = max(clk["vact"],
                                          clk["vpe"] + 100.0) + 996.0
                        deferred.append((slot + 1, 217.0, av_batch))
                        emit_deferred(slot)
                        pull(slot, 2 if u < 12 else 4)

                    # Per-qtile tail on the deferral queue: first a fast DVE
                    # copy of each accumulator to SBUF (the ONLY psum reader,
                    # so the av ring frees ~400ns after the last AV matmul),
                    # then reciprocal+normalize off the copy, then — two
                    # slots later — the PE transposes.
                    state = {}

                    def make_norm(avA=avA, avB=avB, state=state,
                                  final=(u == len(units) - 1)):
                        def norm():
                            recA = nrm.tile([128, 4], f32, tag="recA",
                                            name="recA")
                            recB = nrm.tile([128, 4], f32, tag="recB",
                                            name="recB")
                            # both heads side by side: [q, qc, (A|B)*64] so
                            # each transpose below covers all 128 partitions
                            avn = nrm.tile([128, 4, 128], bf16, tag="avn",
                                           name="avn")
                            if final:
                                # tail path: no successor needs the av ring,
                                # so read psum directly (saves two copies of
                                # latency on the critical path)
                                srcA, srcB = avA, avB
                            else:
                                # SBUF copies are the only psum readers, so
                                # the av ring frees ~400ns after the last AV
                                # matmul instead of after the whole chain
                                srcA = nrm.tile([128, 4, 65], f32, tag="cpA",
                                                name="cpA", bufs=1)
                                srcB = nrm.tile([128, 4, 65], f32, tag="cpB",
                                                name="cpB", bufs=1)
                                nc.vector.tensor_copy(srcA[:], avA[:])
                                nc.vector.tensor_copy(srcB[:], avB[:])
                            nc.vector.reciprocal(recA[:], srcA[:, :, 64])
                            for qc in range(4):
                                nc.vector.tensor_scalar_mul(
                                    avn[:, qc, 0:64], srcA[:, qc, 0:64],
                                    recA[:, qc:qc + 1])
                            nc.vector.reciprocal(recB[:], srcB[:, :, 64])
                            for qc in range(4):
                                nc.vector.tensor_scalar_mul(
                                    avn[:, qc, 64:128], srcB[:, qc, 0:64],
                                    recB[:, qc:qc + 1])
                            state["avn"] = avn
                        return norm

                    def make_transp(pr=pr, t=t, state=state):
                        def transp():
                            finish_cur()   # aot needs sole use of the ring
                            avn = state["avn"]
                            aot = ps.tile([128, 512], bf16, tag="px",
                                          name="aot")
                            # 4 full-partition transposes, one group chain in
                            # the aot bank. Output partitions = avn free dim
                            # = (A hd | B hd) = the head-major g-chunk layout.
                            for qc in range(4):
                                nc.tensor.matmul(
                                    aot[:, qc * 128:(qc + 1) * 128],
                                    avn[:, qc, :], ident_sb[:],
                                    is_transpose=True,
                                    start=(qc == 0), stop=(qc == 3))
                            nc.vector.tensor_copy(
                                AOT_sb[:, pr, t * qw:(t + 1) * qw], aot[:])
                            done_t[t] += 1
                            if done_t[t] == PAIRS:
                                # AOT sn-block t complete: queue outproj
                                for mo in range(d // 128):
                                    oq.append((mo, t))
                        return transp

                    last = u * 16 + 15
                    deferred.append((last + 1, 0.0, make_norm()))
                    deferred.append((last + 3, 220.0, make_transp()))
                    if u == 0:
                        # arm the self-clocked pacing once the DMA-gated
                        # first unit is behind us; resync the PE frontier
                        clk["on"] = True
                        clk["vpe"] = clk["vact"] - 400.0

            # flush the deferral queue (last unit's av batch + tail)
            for es, cost, fn in deferred:
                fn()
            deferred[:] = []

            # ---------------- epilogue ----------------
            # Attention is done: the "sc" psum ring is free, so alternate
            # outproj chunks between the px and sc rings (4 banks of
            # lookahead) to hide the DVE-copy release latency.
            drain(10 ** 6)
            finish_cur()
            ntag = 0
            while oq:
                mo, n0 = oq.popleft()
                for _ in outproj_chunk(mo, n0, ("px", "sc")[ntag % 2]):
                    pass
                ntag += 1

    if not dbg:
        _strip_ldweights(nc)
        _split_multiwaits(nc, cap=1)
    return nc


def _get_nc():
    if "nc" not in _CACHE:
        _CACHE["nc"] = build_mha_nc()
    return _CACHE["nc"]


def make_in_maps(q, k, v, Wq, bq, Wk, bk, Wv, bv, Wo, bo, **_ignored):
    """Shard + lay out the full inputs for the 8 cores."""
    bf = ml_dtypes.bfloat16
    q = np.asarray(q, np.float32)
    k = np.asarray(k, np.float32)
    v = np.asarray(v, np.float32)
    Wq = np.asarray(Wq, np.float32)
    Wk = np.asarray(Wk, np.float32)
    Wv = np.asarray(Wv, np.float32)
    Wo = np.asarray(Wo, np.float32)
    bq = np.asarray(bq, np.float32)
    bk = np.asarray(bk, np.float32)
    bv = np.asarray(bv, np.float32)
    eye = np.eye(128, dtype=bf)

    in_maps = []
    for c in range(8):
        b, gi = divmod(c, 2)
        gs = slice(gi * G, (gi + 1) * G)
        in_maps.append({
            "qT": np.ascontiguousarray(q[b].T).astype(bf),
            "kT": np.ascontiguousarray(k[b].T).astype(bf),
            "vT": np.ascontiguousarray(v[b].T).astype(bf),
            "Wq": np.ascontiguousarray(Wq[:, gs]).astype(bf),
            "Wk": np.ascontiguousarray(Wk[:, gs]).astype(bf),
            "Wv": np.ascontiguousarray(Wv[:, gs]).astype(bf),
            "Wo": np.ascontiguousarray(Wo[gs, :]).astype(bf),
            "bq": np.ascontiguousarray(bq[gs].reshape(G // 128, 128).T),
            "bk": np.ascontiguousarray(bk[gs].reshape(G // 128, 128).T),
            "bv": np.ascontiguousarray(bv[gs][None, :]).astype(bf),
            "ident": eye,
        })
    return in_maps


def run(in_maps, trace=False, trace_kwargs=None):
    from concourse.bass_utils import run_bass_kernel_spmd

    nc = _get_nc()
    kw = {}
    if trace:
        kw["trace"] = True
        kw.update(trace_kwargs or {})
    return run_bass_kernel_spmd(nc, in_maps, core_ids=list(range(8)), **kw)


def kernel(q, k, v, Wq, bq, Wk, bk, Wv, bv, Wo, bo, **_ignored):
    in_maps = make_in_maps(q, k, v, Wq, bq, Wk, bk, Wv, bv, Wo, bo)
    res = run(in_maps)
    bo = np.asarray(bo, np.float32)
    bv64 = np.asarray(bv, np.float64)
    Wo64 = np.asarray(Wo, np.float64)
    # bv rides through the softmax weighting unchanged, so it folds into
    # the output bias exactly: out += bv @ Wo
    bo_eff = (bo.astype(np.float64) + bv64 @ Wo64).astype(np.float32)
    out = np.empty((B, S, D), np.float32)
    for b in range(B):
        acc = (res.results[2 * b]["outT"].astype(np.float32)
               + res.results[2 * b + 1]["outT"].astype(np.float32))
        out[b] = acc.T + bo_eff[None, :]
    return out

